# revision 14
# baseline (speedup 1.0000x reference)
"""ClusterLoss Trainium2 Bass kernel (8-core SPMD).

Problem: features [32768, 1024] f32, 2048 identities x 16 contiguous images.
Returns (cluster_loss scalar, intra_max_distance [2048], inter_min_distance [2048]).

Sharding: data-parallel over identities (256 ids / core). Each core:
  - computes its centers + intra-max distances locally (f32),
  - PE-transposes its centers, AllGathers an fp16 [1026, 256] payload
    (rows 0..1023 = centers^T, rows 1024/1025 = ||c||^2 split hi/lo),
  - computes w_ij = <c_i, c_j> - 0.5*c2_j for its 256 rows x all 2048 cols
    via fp16 PE matmuls accumulating in f32 PSUM (augmented contraction rows
    add the c2_j term), then v = -2*w + BIG*diag, row-min, + c2_i, sqrt.
Host only shards inputs, concatenates outputs, and computes the final
mean(relu(intra - inter + margin)) over the returned [2048] vectors.
"""

import numpy as np

import concourse.bass as bass
import concourse.mybir as mybir
import concourse.tile as tile
from concourse import bacc, masks
from concourse.bass_utils import run_bass_kernel_spmd

# Problem constants (hardcoded per spec)
L = 2048          # identities
K = 16            # images per identity
D = 1024          # feature dim
N = L * K         # 32768 rows
NCORES = 8
IDS = L // NCORES  # 256 ids per core
G = 2              # id groups of 128 (partition dim)
H = 2              # halves of K (8 images each)
KH = K // H        # 8
MARGIN = 10.0
EPS = 1e-12
BIG = 1e30

F32 = mybir.dt.float32
F16 = mybir.dt.float16
AX = mybir.AxisListType
ALU = mybir.AluOpType
ACTF = mybir.ActivationFunctionType

PAYROWS = D + 2    # 1026: centers^T rows + c2 hi row + c2 lo row


def _emit_rep(nc, tc, pools, io, rep, prev_dep):
    """Emit one full cluster-loss computation. Returns a tile whose write
    completes only after this rep's last result (used to chain reps when
    benchmarking)."""
    (fp, redp, scrp, maskp, rhsp, smallp, pp, dram) = pools
    (feat, fv, negoff, out_intra, out_inter, ident, iota_t, nofft) = io

    centers = [
        pp.tile([128, D], F32, name=f"centers{g}_{rep}", tag=f"centers{g}") for g in range(G)
    ]
    c2g = [pp.tile([128, 1], F32, name=f"c2g{g}_{rep}", tag=f"c2g{g}") for g in range(G)]
    d2 = [pp.tile([128, K], F32, name=f"d2g{g}_{rep}", tag=f"d2g{g}") for g in range(G)]
    lhsT = [
        pp.tile([128, IDS], F16, name=f"lhsT{dc}_{rep}", tag=f"lhsT{dc}") for dc in range(8)
    ]

    # augmented contraction rows add -0.5*c2_j so the post-step's (-2)
    # scale yields v = c2_j - 2*cc
    lhsT9 = pp.tile([128, 128], F16, name=f"lhsT9_{rep}", tag="lhsT9")
    nc.gpsimd.memset(lhsT9[:], 0.0)
    nc.gpsimd.memset(lhsT9[0:2, :], -0.5)

    c2row_sb = pp.tile([1, IDS], F32, name=f"c2row_sb_{rep}", tag="c2row_sb")
    c2hi_sb = pp.tile([1, IDS], F16, name=f"c2hi_sb_{rep}", tag="c2hi_sb")
    c2lo_f32 = pp.tile([1, IDS], F32, name=f"c2lo_f32_{rep}", tag="c2lo_f32")
    c2lo_sb = pp.tile([1, IDS], F16, name=f"c2lo_sb_{rep}", tag="c2lo_sb")

    pay = dram.tile([PAYROWS, IDS], F16, name=f"pay_{rep}", tag="pay")
    agout = dram.tile(
        [NCORES * PAYROWS, IDS], F16, addr_space="Shared", name=f"agout_{rep}"
    )

    ftiles = {}
    for g in range(G):
        # ---- load features + centers ----
        sums = []
        for h in range(H):
            ft = fp.tile([128, KH * D], F32, name="ft", tag="ft")
            if prev_dep is not None:
                # zero-valued WAW gate: forces this rep's loads after the
                # previous rep's final result (serial timing for benchmarks)
                nc.vector.scalar_tensor_tensor(
                    out=ft[:, 0:1], in0=prev_dep[:], scalar=0.0,
                    in1=prev_dep[:], op0=ALU.mult, op1=ALU.mult,
                )
            nc.sync.dma_start(out=ft[:], in_=fv[g, h])
            ftiles[(g, h)] = ft
            red = redp.tile([128, D], F32, name="red", tag="red")
            nc.vector.tensor_reduce(
                out=red[:],
                in_=ft[:].rearrange("p (k d) -> p d k", k=KH),
                axis=AX.X,
                op=ALU.add,
            )
            sums.append(red)
        nc.vector.tensor_tensor(
            out=sums[0][:], in0=sums[0][:], in1=sums[1][:], op=ALU.add
        )
        nc.scalar.activation(
            out=centers[g][:], in_=sums[0][:], func=ACTF.Copy, scale=1.0 / K
        )

        # ---- c2 = ||center||^2 ----
        scr2 = scrp.tile([128, D], F32, name="scr", tag="scr")
        nc.scalar.activation(
            out=scr2[:], in_=centers[g][:], func=ACTF.Square,
            accum_out=c2g[g][:],
        )

        # ---- intra: diff (in place), square+accum per image, max, sqrt ----
        for h in range(H):
            ft = ftiles[(g, h)]
            ftv = ft[:].rearrange("p (k d) -> p k d", k=KH)
            cb = centers[g][:][:, None, :].broadcast_to([128, KH, D])
            nc.vector.tensor_tensor(out=ftv, in0=ftv, in1=cb, op=ALU.subtract)
            for k in range(KH):
                scr = scrp.tile([128, D], F32, name="scr", tag="scr")
                col = h * KH + k
                nc.scalar.activation(
                    out=scr[:],
                    in_=ft[:, k * D:(k + 1) * D],
                    func=ACTF.Square,
                    accum_out=d2[g][:, col:col + 1],
                )
        dmax = smallp.tile([128, 1], F32, name="dmax", tag="dmax")
        nc.vector.tensor_reduce(
            out=dmax[:], in_=d2[g][:], axis=AX.X, op=ALU.max
        )
        nc.vector.tensor_scalar_max(dmax[:], dmax[:], EPS)
        intra_sb = smallp.tile([128, 1], F32, name="intra_sb", tag="intra_sb")
        nc.scalar.activation(out=intra_sb[:], in_=dmax[:], func=ACTF.Sqrt)
        nc.sync.dma_start(out=out_intra[g], in_=intra_sb[:])

    # ---- transpose centers -> lhsT (SBUF fp16) -> payload ----
    with tc.tile_pool(name=f"pst{rep}", bufs=2, space="PSUM") as pstp:
        for g in range(G):
            for dc in range(8):
                ps = pstp.tile([128, 128], F32, name="ps", tag="ps", space="PSUM")
                nc.tensor.transpose(
                    ps[:], centers[g][:, dc * 128:(dc + 1) * 128], ident[:]
                )
                nc.scalar.activation(
                    out=lhsT[dc][:, g * 128:(g + 1) * 128], in_=ps[:],
                    func=ACTF.Copy,
                )
                nc.sync.dma_start(
                    out=pay[dc * 128:(dc + 1) * 128, g * 128:(g + 1) * 128],
                    in_=lhsT[dc][:, g * 128:(g + 1) * 128],
                )
            psc = pstp.tile([1, 128], F32, name="psc", tag="ps", space="PSUM")
            nc.tensor.transpose(psc[:], c2g[g][:], ident[:])
            nc.scalar.activation(
                out=c2row_sb[:, g * 128:(g + 1) * 128], in_=psc[:],
                func=ACTF.Copy,
            )
    # c2 split into fp16 hi + lo so the matmul reconstructs it exactly
    nc.scalar.activation(out=c2hi_sb[:], in_=c2row_sb[:], func=ACTF.Copy)
    nc.vector.tensor_tensor(
        out=c2lo_f32[:], in0=c2row_sb[:], in1=c2hi_sb[:], op=ALU.subtract
    )
    nc.vector.tensor_copy(c2lo_sb[:], c2lo_f32[:])
    nc.sync.dma_start(out=pay[D:D + 1, :], in_=c2hi_sb[:])
    nc.sync.dma_start(out=pay[D + 1:D + 2, :], in_=c2lo_sb[:])

    # ---- AllGather payload ----
    nc.gpsimd.collective_compute(
        "AllGather",
        ALU.bypass,
        replica_groups=[list(range(NCORES))],
        ins=[pay.opt()],
        outs=[agout.opt()],
    )

    # ---- read back gathered payload ----
    agv = agout.rearrange("(c r) j -> r c j", c=NCORES)
    rhs9 = pp.tile([128, L], F16, name=f"rhs9_{rep}", tag="rhs9")
    nc.gpsimd.memset(rhs9[:], 0.0)
    nc.sync.dma_start(
        out=rhs9[0:2, :].rearrange("p (c j) -> p c j", c=NCORES),
        in_=agv[D:D + 2],
    )
    rhs = []
    for dc in range(8):
        rt = rhsp.tile([128, L], F16, name="rt", tag="rt")
        nc.sync.dma_start(
            out=rt[:].rearrange("p (c j) -> p c j", c=NCORES),
            in_=agv[dc * 128:(dc + 1) * 128],
        )
        rhs.append(rt)

    # ---- w = cc - 0.5*c2_j : accumulate 9 K-chunks into f32 PSUM ----
    inter_done = None
    with tc.tile_pool(name=f"vp{rep}", bufs=2, space="PSUM") as vpp:
        vps = [
            vpp.tile([128, L], F32, name=f"vps{g}", tag="vps") for g in range(G)
        ]
        for kc in range(9):
            for g in range(G):
                if kc < 8:
                    lt = lhsT[kc][:, g * 128:(g + 1) * 128]
                    rsrc = rhs[kc]
                else:
                    lt = lhsT9[:]
                    rsrc = rhs9
                for nch in range(4):
                    nc.tensor.matmul(
                        vps[g][:, nch * 512:(nch + 1) * 512],
                        lhsT=lt,
                        rhs=rsrc[:, nch * 512:(nch + 1) * 512],
                        start=(kc == 0),
                        stop=(kc == 8),
                    )

        # ---- v = -2*w + BIG*diag ; min ; + c2_i ; sqrt ----
        for g in range(G):
            mask = maskp.tile([128, L], F32, name="mask", tag="mask")
            nc.vector.tensor_scalar(
                mask[:], iota_t[:], nofft[:, g:g + 1], BIG, ALU.is_equal,
                ALU.mult,
            )
            nc.vector.scalar_tensor_tensor(
                out=vps[g][:], in0=vps[g][:], scalar=-2.0, in1=mask[:],
                op0=ALU.mult, op1=ALU.add,
            )
            minv = smallp.tile([128, 1], F32, name="minv", tag="minv")
            nc.vector.tensor_reduce(
                out=minv[:], in_=vps[g][:], axis=AX.X, op=ALU.min
            )
            nc.vector.tensor_tensor(
                out=minv[:], in0=minv[:], in1=c2g[g][:], op=ALU.add
            )
            nc.vector.tensor_scalar_max(minv[:], minv[:], EPS)
            inter_sb = smallp.tile(
                [128, 1], F32, name="inter_sb", tag="inter_sb"
            )
            nc.scalar.activation(out=inter_sb[:], in_=minv[:], func=ACTF.Sqrt)
            nc.sync.dma_start(out=out_inter[g], in_=inter_sb[:])
            inter_done = inter_sb
    return inter_done


def build_nc(reps=1):
    nc = bacc.Bacc(
        "TRN2",
        target_bir_lowering=False,
        debug=False,
        num_devices=NCORES,
    )

    feat = nc.dram_tensor("features", [N // NCORES, D], F32, kind="ExternalInput")
    negoff = nc.dram_tensor("negoff", [G, 128, 1], F32, kind="ExternalInput")
    out_intra = nc.dram_tensor("out_intra", [G, 128, 1], F32, kind="ExternalOutput")
    out_inter = nc.dram_tensor("out_inter", [G, 128, 1], F32, kind="ExternalOutput")

    # row = (g*128 + p)*16 + h*8 + k  ->  [g, h, p, (k d)]
    fv = feat.rearrange("(g p h k) d -> g h p (k d)", g=G, p=128, h=H, k=KH)

    with tile.TileContext(nc) as tc:
        with (
            tc.tile_pool(name="fp", bufs=3) as fp,
            tc.tile_pool(name="redp", bufs=2) as redp,
            tc.tile_pool(name="scrp", bufs=2) as scrp,
            tc.tile_pool(name="maskp", bufs=2) as maskp,
            tc.tile_pool(name="rhsp", bufs=4) as rhsp,
            tc.tile_pool(name="smallp", bufs=2) as smallp,
            tc.tile_pool(name="persist", bufs=1) as pp,
            tc.tile_pool(name="dram", bufs=1, space="DRAM") as dram,
        ):
            ident = pp.tile([128, 128], F32, name="ident")
            masks.make_identity(nc, ident[:])

            iota_t = pp.tile([128, L], F32, name="iota_t")
            # iota[p, j] = p - j ; diagonal of group (c, g) is where
            # p - j == -(c*256 + g*128).  Small integers, exact in f32.
            nc.gpsimd.iota(
                iota_t[:], pattern=[[-1, L]], base=0, channel_multiplier=1,
                allow_small_or_imprecise_dtypes=True,
            )

            nofft = pp.tile([128, G], F32, name="nofft")
            nc.sync.dma_start(
                out=nofft[:], in_=negoff[:, :, 0].rearrange("g p -> p g")
            )

            pools = (fp, redp, scrp, maskp, rhsp, smallp, pp, dram)
            io = (feat, fv, negoff, out_intra, out_inter, ident, iota_t, nofft)

            prev = None
            for rep in range(reps):
                prev = _emit_rep(nc, tc, pools, io, rep, prev)

    nc.compile()
    return nc


_CACHE = {}


def _get_nc(reps=1):
    key = f"nc{reps}"
    if key not in _CACHE:
        _CACHE[key] = build_nc(reps)
    return _CACHE[key]


def make_in_maps(features: np.ndarray):
    features = np.asarray(features, dtype=np.float32)
    in_maps = []
    rows = N // NCORES
    for c in range(NCORES):
        sh = np.ascontiguousarray(features[c * rows:(c + 1) * rows])
        noff = np.zeros((G, 128, 1), np.float32)
        for g in range(G):
            noff[g, :, :] = -(c * IDS + g * 128)
        in_maps.append({"features": sh, "negoff": noff})
    return in_maps


def kernel(features, targets=None, **unused):
    nc = _get_nc()
    in_maps = make_in_maps(features)
    res = run_bass_kernel_spmd(nc, in_maps, core_ids=list(range(NCORES)))
    intra = np.concatenate(
        [res.results[c]["out_intra"].reshape(IDS) for c in range(NCORES)]
    ).astype(np.float32)
    inter = np.concatenate(
        [res.results[c]["out_inter"].reshape(IDS) for c in range(NCORES)]
    ).astype(np.float32)
    loss = np.float32(
        np.mean(np.maximum(intra - inter + np.float32(MARGIN), np.float32(0.0)))
    )
    return loss, intra, inter


# revision 28
# speedup vs baseline: 1.0465x; 1.0465x over previous
"""ClusterLoss Trainium2 Bass kernel (8-core SPMD).

Problem: features [32768, 1024] f32, 2048 identities x 16 contiguous images.
Returns (cluster_loss scalar, intra_max_distance [2048], inter_min_distance [2048]).

Sharding: data-parallel over identities (256 ids / core). Each core:
  - computes its centers + intra-max distances locally (f32),
  - PE-transposes its centers, AllGathers an fp16 [1026, 256] payload
    (rows 0..1023 = centers^T, rows 1024/1025 = ||c||^2 split hi/lo),
  - computes w_ij = <c_i, c_j> - 0.5*c2_j for its 256 rows x all 2048 cols
    via fp16 PE matmuls accumulating in f32 PSUM (augmented contraction rows
    add the c2_j term), then v = -2*w + BIG*diag, row-min, + c2_i, sqrt.
Host only shards inputs, concatenates outputs, and computes the final
mean(relu(intra - inter + margin)) over the returned [2048] vectors.

Phase structure (critical path first): feature DMA -> center reduces ->
transposes -> AllGather -> rhs loads -> matmuls -> min/post. The intra
diff/square work overlaps the AllGather window. DMA issue is spread across
sequencers (SP: features, ACT: payload + outputs, PE: rhs) to avoid
head-of-line blocking on one in-order queue.
"""

import numpy as np

import concourse.bass as bass
import concourse.mybir as mybir
import concourse.tile as tile
from concourse import bacc, masks
from concourse.bass_utils import run_bass_kernel_spmd

# Problem constants (hardcoded per spec)
L = 2048          # identities
K = 16            # images per identity
D = 1024          # feature dim
N = L * K         # 32768 rows
NCORES = 8
IDS = L // NCORES  # 256 ids per core
G = 2              # id groups of 128 (partition dim)
H = 2              # halves of K (8 images each)
KH = K // H        # 8
MARGIN = 10.0
EPS = 1e-12
BIG = 57344.0      # >> any center distance^2; BIG/2 = 28672 exact in fp8e5

F32 = mybir.dt.float32
F16 = mybir.dt.float16
F8 = mybir.dt.float8e5
AX = mybir.AxisListType
ALU = mybir.AluOpType
ACTF = mybir.ActivationFunctionType

PAYROWS = D + 2    # 1026: centers^T rows + c2 hi row + c2 lo row


def _emit_rep(nc, tc, pools, io, rep, prev_dep, n_cores=NCORES):
    """Emit one full cluster-loss computation. Returns a tile whose write
    completes only after this rep's last result (used to chain reps when
    benchmarking)."""
    from concourse.tile_rust import add_dep_helper

    (fp, redp, maskp, rhsp, smallp, pp, dram) = pools
    (feat, fv, dbias, out_intra, out_inter, ident) = io

    centers = [
        pp.tile([128, D], F32, name=f"centers{g}_{rep}", tag=f"centers{g}")
        for g in range(G)
    ]
    redacc = [
        pp.tile([128, D], F32, name=f"redacc{g}_{rep}", tag=f"redacc{g}")
        for g in range(G)
    ]
    c2g = [
        pp.tile([128, 1], F32, name=f"c2g{g}_{rep}", tag=f"c2g{g}")
        for g in range(G)
    ]
    d2 = [
        pp.tile([128, K], F32, name=f"d2g{g}_{rep}", tag=f"d2g{g}")
        for g in range(G)
    ]
    # all 8 transposed-center chunks in one tile: chunk dc occupies
    # columns [dc*256, dc*256+256) with the two g-halves inside
    lhsT_all = pp.tile([128, 8 * IDS], F16, name=f"lhsT_{rep}", tag="lhsT")

    # augmented contraction rows add -0.5*c2_j so the post-step's (-2)
    # scale yields v = c2_j - 2*cc
    lhsT9 = pp.tile([128, 128], F16, name=f"lhsT9_{rep}", tag="lhsT9")
    nc.gpsimd.memset(lhsT9[:], 0.0)
    nc.gpsimd.memset(lhsT9[0:2, :], -0.5)

    c2row_sb = pp.tile([1, IDS], F32, name=f"c2row_sb_{rep}", tag="c2row_sb")
    c2hi_sb = pp.tile([1, IDS], F16, name=f"c2hi_sb_{rep}", tag="c2hi_sb")
    c2lo_f32 = pp.tile([1, IDS], F32, name=f"c2lo_f32_{rep}", tag="c2lo_f32")
    c2lo_sb = pp.tile([1, IDS], F16, name=f"c2lo_sb_{rep}", tag="c2lo_sb")

    dbias_sb = pp.tile([128, G * L], F8, name=f"dbias_sb_{rep}", tag="dbias_sb")
    for g in range(G):
        nc.sync.dma_start(out=dbias_sb[:, g * L:(g + 1) * L], in_=dbias[g])
    ident8 = pp.tile([128, 128], F8, name=f"ident8_{rep}", tag="ident8")
    nc.scalar.activation(out=ident8[:], in_=ident[:], func=ACTF.Copy)

    # payload rows 0..127: lhsT_all dump; row 128/129: c2 hi/lo in cols 0:256
    pay = dram.tile([130, 8 * IDS], F16, name=f"pay_{rep}", tag="pay")
    agout = dram.tile(
        [NCORES * 130, 8 * IDS], F16, addr_space="Shared", name=f"agout_{rep}"
    )

    # ============ Phase A: loads, centers (quarter-granular) ==============
    Q = 4  # quarter tiles of 4 images each
    KQ = K // Q
    ftiles = {}
    adds = {}
    with tc.tile_pool(name=f"redtmp{rep}", bufs=2, space="PSUM") as redtmp:
        for g in range(G):
            for q in range(Q):
                ft = fp.tile([128, KQ * D], F32, name="ft", tag="ft")
                if prev_dep is not None:
                    # zero-valued WAW gate: forces this rep's loads after the
                    # previous rep's final result (serial benchmark timing)
                    nc.vector.scalar_tensor_tensor(
                        out=ft[:, 0:1], in0=prev_dep[:], scalar=0.0,
                        in1=prev_dep[:], op0=ALU.mult, op1=ALU.mult,
                    )
                nc.sync.dma_start(out=ft[:], in_=fv[g, q])
                ftiles[(g, q)] = ft
                if q == 0:
                    nc.vector.tensor_reduce(
                        out=redacc[g][:],
                        in_=ft[:].rearrange("p (k d) -> p d k", k=KQ),
                        axis=AX.X,
                        op=ALU.add,
                    )
                else:
                    rt = redtmp.tile([128, D], F32, name="rt", tag="rt",
                                     space="PSUM", bufs=1)
                    nc.vector.tensor_reduce(
                        out=rt[:],
                        in_=ft[:].rearrange("p (k d) -> p d k", k=KQ),
                        axis=AX.X,
                        op=ALU.add,
                    )
                    a = nc.vector.tensor_tensor(
                        out=redacc[g][:], in0=redacc[g][:], in1=rt[:],
                        op=ALU.add,
                    )
                    adds[(g, q)] = a
            nc.scalar.activation(
                out=centers[g][:], in_=redacc[g][:], func=ACTF.Copy,
                scale=1.0 / K,
            )
            # c2 = ||center||^2 (scratch output goes to PSUM, value unused)
            scr2 = redtmp.tile([128, D], F32, name="scr", tag="scr",
                               space="PSUM", bufs=1)
            nc.scalar.activation(
                out=scr2[:], in_=centers[g][:], func=ACTF.Square,
                accum_out=c2g[g][:],
            )
            # c2 hi/lo fp16 split (column form, used for payload rows)
            hi = smallp.tile([128, 1], F16, name="c2hic", tag="c2hic")
            nc.scalar.activation(out=hi[:], in_=c2g[g][:], func=ACTF.Copy)
            lo = smallp.tile([128, 1], F32, name="c2loc", tag="c2loc")
            nc.vector.tensor_tensor(
                out=lo[:], in0=c2g[g][:], in1=hi[:], op=ALU.subtract
            )
            hi32 = smallp.tile([128, 1], F32, name="c2hic32", tag="c2hic32")
            nc.scalar.activation(out=hi32[:], in_=hi[:], func=ACTF.Copy)

            # transpose centers -> lhsT_all (fp16)
            with tc.tile_pool(name=f"pst{rep}_{g}", bufs=4,
                              space="PSUM") as pstp:
                for dc in range(8):
                    ps = pstp.tile([128, 128], F32, name="ps", tag="ps",
                                   space="PSUM")
                    nc.tensor.transpose(
                        ps[:], centers[g][:, dc * 128:(dc + 1) * 128],
                        ident[:],
                    )
                    dst = lhsT_all[:, dc * IDS + g * 128:
                                   dc * IDS + (g + 1) * 128]
                    nc.scalar.activation(out=dst, in_=ps[:], func=ACTF.Copy)
                # c2 hi/lo rows via PE transpose of the column forms
                psh = pstp.tile([1, 128], F32, name="psh", tag="ps",
                                space="PSUM")
                nc.tensor.transpose(psh[:], hi32[:], ident[:])
                nc.scalar.activation(
                    out=c2hi_sb[:, g * 128:(g + 1) * 128], in_=psh[:],
                    func=ACTF.Copy,
                )
                psl = pstp.tile([1, 128], F32, name="psl", tag="ps",
                                space="PSUM")
                nc.tensor.transpose(psl[:], lo[:], ident[:])
                nc.scalar.activation(
                    out=c2lo_sb[:, g * 128:(g + 1) * 128], in_=psl[:],
                    func=ACTF.Copy,
                )

    # payload: one big dump + two c2 rows, then AllGather
    nc.scalar.dma_start(out=pay[0:128, :], in_=lhsT_all[:])
    nc.scalar.dma_start(out=pay[128:129, 0:IDS], in_=c2hi_sb[:])
    nc.scalar.dma_start(out=pay[129:130, 0:IDS], in_=c2lo_sb[:])

    if n_cores > 1:
        nc.gpsimd.collective_compute(
            "AllGather",
            ALU.bypass,
            replica_groups=[list(range(n_cores))],
            ins=[pay.opt()],
            outs=[agout.opt()],
        )
    else:
        # collective-free variant for cost-model timeline analysis
        nc.sync.dma_start(out=agout[0:130, :], in_=pay)

    # ============ Phase B: intra diff/square work (overlaps AllGather) ====
    last_add = adds[(G - 1, Q - 1)]
    for g in range(G):
        for q in range(Q):
            ft = ftiles[(g, q)]
            ftv = ft[:].rearrange("p (k d) -> p k d", k=KQ)
            cb = centers[g][:][:, None, :].broadcast_to([128, KQ, D])
            di = nc.vector.tensor_tensor(
                out=ftv, in0=ftv, in1=cb, op=ALU.subtract
            )
            # keep the center-reduce chain ahead of diffs on DVE
            add_dep_helper(di.ins, last_add.ins, sync=False,
                           reason="diffs after center reduces")
            for k in range(KQ):
                col = q * KQ + k
                nc.scalar.activation(
                    out=ft[:, k * D:(k + 1) * D],
                    in_=ft[:, k * D:(k + 1) * D],
                    func=ACTF.Square,
                    accum_out=d2[g][:, col:col + 1],
                )
        dmax = smallp.tile([128, 1], F32, name="dmax", tag="dmax")
        nc.vector.tensor_reduce(
            out=dmax[:], in_=d2[g][:], axis=AX.X, op=ALU.max
        )
        nc.vector.tensor_scalar_max(dmax[:], dmax[:], EPS)
        intra_sb = smallp.tile([128, 1], F32, name="intra_sb", tag="intra_sb")
        nc.scalar.activation(out=intra_sb[:], in_=dmax[:], func=ACTF.Sqrt)
        nc.scalar.dma_start(out=out_intra[g], in_=intra_sb[:])

    # ============ Phase C: rhs readback, matmuls, min/post ================
    # agout row c*130 + p, col dc*256 + idl  ==  C_c[idl, dc*128 + p]
    agr = agout.rearrange("(c r) j -> r c j", c=NCORES)
    rhs9 = pp.tile([2, L], F16, name=f"rhs9_{rep}", tag="rhs9")
    for i in range(2):
        nc.gpsimd.dma_start(
            out=rhs9[i:i + 1, :].rearrange("p (c j) -> p c j", c=NCORES),
            in_=agr[128 + i][:, None, 0:IDS],
        )
    # single rhs tile [128, 8*2048]: chunk dc at columns [dc*2048, ...)
    rhs_all = rhsp.tile([128, 8 * L], F16, name="rhs_all", tag="rhs_all")
    for dc in range(8):
        nc.sync.dma_start(
            out=rhs_all[:, dc * L:(dc + 1) * L].rearrange(
                "p (c j) -> p c j", c=NCORES
            ),
            in_=agr[0:128, :, dc * IDS:(dc + 1) * IDS],
        )

    # w = cc - 0.5*c2_j ; accumulate into f32 PSUM. kc-outer so matmuls
    # start as soon as each rhs chunk lands; the augmented c2 chunk plus
    # the mask/min post-processing run per 512-column block at the end.
    inter_done = None
    minp = [
        smallp.tile([128, 4], F32, name=f"minp{g}_{rep}", tag=f"minp{g}")
        for g in range(G)
    ]
    with tc.tile_pool(name=f"vp{rep}", bufs=2, space="PSUM") as vpp:
        vps = [
            vpp.tile([128, L], F32, name=f"vps{g}", tag="vps")
            for g in range(G)
        ]
        # preload psum with -BIG/2 on the diagonal via an identity-weighted
        # fp8 matmul (PE is idle here; matmuls then accumulate w on top, so
        # the diagonal can never win the max)
        for g in range(G):
            for nch in range(4):
                nc.tensor.matmul(
                    vps[g][:, nch * 512:(nch + 1) * 512],
                    lhsT=ident8[:],
                    rhs=dbias_sb[:, g * L + nch * 512:
                                 g * L + (nch + 1) * 512],
                    start=True,
                    stop=False,
                )
        for kc in range(8):
            for g in range(G):
                lt = lhsT_all[:, kc * IDS + g * 128: kc * IDS + (g + 1) * 128]
                for nch in range(4):
                    nc.tensor.matmul(
                        vps[g][:, nch * 512:(nch + 1) * 512],
                        lhsT=lt,
                        rhs=rhs_all[:, kc * L + nch * 512:
                                    kc * L + (nch + 1) * 512],
                        start=False,
                        stop=False,
                        skip_group_check=True,
                    )
        for nch in range(4):
            for g in range(G):
                nc.tensor.matmul(
                    vps[g][:, nch * 512:(nch + 1) * 512],
                    lhsT=lhsT9[0:2, :],
                    rhs=rhs9[:, nch * 512:(nch + 1) * 512],
                    start=False,
                    stop=True,
                    skip_group_check=True,
                )
            # row-max of w on this block; min_j v = -2 * max_j w
            for g in range(G):
                vslice = vps[g][:, nch * 512:(nch + 1) * 512]
                nc.vector.tensor_reduce(
                    out=minp[g][:, nch:nch + 1], in_=vslice, axis=AX.X,
                    op=ALU.max,
                )

        for g in range(G):
            minv = smallp.tile([128, 1], F32, name="minv", tag="minv")
            nc.vector.tensor_reduce(
                out=minv[:], in_=minp[g][:], axis=AX.X, op=ALU.max
            )
            # inter^2 = c2_i - 2 * max_j w
            nc.vector.scalar_tensor_tensor(
                out=minv[:], in0=minv[:], scalar=-2.0, in1=c2g[g][:],
                op0=ALU.mult, op1=ALU.add,
            )
            nc.vector.tensor_scalar_max(minv[:], minv[:], EPS)
            inter_sb = smallp.tile(
                [128, 1], F32, name="inter_sb", tag="inter_sb"
            )
            nc.scalar.activation(out=inter_sb[:], in_=minv[:], func=ACTF.Sqrt)
            nc.scalar.dma_start(out=out_inter[g], in_=inter_sb[:])
            inter_done = inter_sb
    return inter_done


def build_nc(reps=1, n_cores=NCORES):
    nc = bacc.Bacc(
        "TRN2",
        target_bir_lowering=False,
        debug=False,
        num_devices=n_cores,
    )

    feat = nc.dram_tensor("features", [N // NCORES, D], F32, kind="ExternalInput")
    dbias = nc.dram_tensor("dbias", [G, 128, L], F8, kind="ExternalInput")
    out_intra = nc.dram_tensor("out_intra", [G, 128, 1], F32, kind="ExternalOutput")
    out_inter = nc.dram_tensor("out_inter", [G, 128, 1], F32, kind="ExternalOutput")

    # row = (g*128 + p)*16 + q*4 + k  ->  [g, q, p, (k d)]
    fv = feat.rearrange("(g p q k) d -> g q p (k d)", g=G, p=128, q=4, k=4)

    with tile.TileContext(nc) as tc:
        with (
            tc.tile_pool(name="fp", bufs=8) as fp,
            tc.tile_pool(name="redp", bufs=1) as redp,
            tc.tile_pool(name="maskp", bufs=1) as maskp,
            tc.tile_pool(name="rhsp", bufs=1) as rhsp,
            tc.tile_pool(name="smallp", bufs=2) as smallp,
            tc.tile_pool(name="persist", bufs=1) as pp,
            tc.tile_pool(name="dram", bufs=1, space="DRAM") as dram,
        ):
            ident = pp.tile([128, 128], F32, name="ident")
            masks.make_identity(nc, ident[:])

            pools = (fp, redp, maskp, rhsp, smallp, pp, dram)
            io = (feat, fv, dbias, out_intra, out_inter, ident)

            prev = None
            for rep in range(reps):
                prev = _emit_rep(nc, tc, pools, io, rep, prev, n_cores)

    nc.compile()
    return nc


_CACHE = {}


def _get_nc(reps=1, n_cores=NCORES):
    key = f"nc{reps}_{n_cores}"
    if key not in _CACHE:
        _CACHE[key] = build_nc(reps, n_cores)
    return _CACHE[key]


def make_in_maps(features: np.ndarray):
    features = np.asarray(features, dtype=np.float32)
    in_maps = []
    rows = N // NCORES
    for c in range(NCORES):
        sh = np.ascontiguousarray(features[c * rows:(c + 1) * rows])
        import ml_dtypes
        db = np.zeros((G, 128, L), ml_dtypes.float8_e5m2)
        for g in range(G):
            off = c * IDS + g * 128
            db[g, np.arange(128), off + np.arange(128)] = -BIG / 2
        in_maps.append({"features": sh, "dbias": db})
    return in_maps


def kernel(features, targets=None, **unused):
    nc = _get_nc()
    in_maps = make_in_maps(features)
    res = run_bass_kernel_spmd(nc, in_maps, core_ids=list(range(NCORES)))
    intra = np.concatenate(
        [res.results[c]["out_intra"].reshape(IDS) for c in range(NCORES)]
    ).astype(np.float32)
    inter = np.concatenate(
        [res.results[c]["out_inter"].reshape(IDS) for c in range(NCORES)]
    ).astype(np.float32)
    loss = np.float32(
        np.mean(np.maximum(intra - inter + np.float32(MARGIN), np.float32(0.0)))
    )
    return loss, intra, inter
